# revision 1
# baseline (speedup 1.0000x reference)
"""Trainium2 Bass kernel for nn_KernelConv (per-pixel dynamic 5x5 convolution).

  out[b,n,y,x] = W[b,n,y,x] * sum_{i,j} core[b, n*25+i*5+j, y, x] * frames_pad[b, n, y+i-2, x+j-2]

Sharding: pure data parallel. The 16 (b,n) slices are split 2-per-core across
8 NeuronCores; each core runs the same NEFF on its own slice pair.

Per-core program (per slice s in {0,1}, per 128-row strip t in {0..3}):
  - one fused DMA loads 5 overlapping 128-row windows of the (host-padded)
    frames plane into FW[p, i*516 + c] = fpad[y0+i+p, c]
  - one DMA loads the 25 core planes for the strip as C[p, q*512 + x]
  - 5 in-place product ops (one per kernel row i): C[:, (i*5+j)*512+x] *=
    FW[:, i*516 + j + x] using an overlapping (stride-1, stride-1) access
    pattern for the j dimension; split between VectorE and GpSimd
  - one strided tensor_reduce sums the 25 planes (q innermost, stride 512)
  - multiply by W, DMA the strip back out
"""

import numpy as np

import concourse.bacc as bacc
import concourse.bass as bass
import concourse.mybir as mybir
import concourse.tile as tile

F32 = mybir.dt.float32

B, N, H, Wd = 2, 8, 512, 512
K = 5
K2 = K * K
P = 128                      # strip height (SBUF partitions)
NSTRIP = H // P              # 4 strips per slice
SLICES_PER_CORE = 2          # 16 (b,n) slices / 8 cores
HP = H + 4                   # padded frame height
WP = Wd + 4                  # padded frame width
N_CORES = 8

# Which kernel-row products run on VectorE; the rest go to GpSimd.
DVE_ROWS = (0, 1)

_RUNNER = None


def _build_program():
    nc = bacc.Bacc("TRN2", target_bir_lowering=False)
    fp_d = nc.dram_tensor("framesp", (SLICES_PER_CORE, HP, WP), F32, kind="ExternalInput")
    core_d = nc.dram_tensor("corex", (SLICES_PER_CORE, K2, H, Wd), F32, kind="ExternalInput")
    w_d = nc.dram_tensor("w", (SLICES_PER_CORE, H, Wd), F32, kind="ExternalInput")
    out_d = nc.dram_tensor("out", (SLICES_PER_CORE, H, Wd), F32, kind="ExternalOutput")

    with tile.TileContext(nc) as tc:
        with tc.tile_pool(name="sbuf", bufs=2) as pool:
            for s in range(SLICES_PER_CORE):
                for t in range(NSTRIP):
                    y0 = t * P
                    ct = pool.tile([P, K2 * Wd], F32, tag="C")
                    fw = pool.tile([P, K * WP], F32, tag="FW")
                    wt = pool.tile([P, Wd], F32, tag="Wt")
                    acc = pool.tile([P, Wd], F32, tag="acc")

                    # core chunk: [p, q, x] <- core[s, q, y0+p, x]
                    c_src = core_d[s, :, y0:y0 + P, :].transpose([1, 0, 2])
                    c_dst = bass.AP(ct.tensor, ct.offset,
                                    [ct.ap[0], (Wd, K2), (1, Wd)])
                    nc.sync.dma_start(out=c_dst, in_=c_src)

                    # frames windows: FW[p, i*WP + c] = fpad[s, y0+i+p, c]
                    f_src = bass.AP(fp_d.ap().tensor, s * HP * WP + y0 * WP,
                                    [(WP, P), (WP, K), (1, WP)])
                    f_dst = bass.AP(fw.tensor, fw.offset,
                                    [fw.ap[0], (WP, K), (1, WP)])
                    nc.sync.dma_start(out=f_dst, in_=f_src)

                    nc.sync.dma_start(out=wt, in_=w_d[s, y0:y0 + P, :])

                    # products, in place into C: for row i, (j, x) block
                    for i in range(K):
                        c_blk = bass.AP(ct.tensor, ct.offset + i * K * Wd,
                                        [ct.ap[0], (Wd, K), (1, Wd)])
                        f_blk = bass.AP(fw.tensor, fw.offset + i * WP,
                                        [fw.ap[0], (1, K), (1, Wd)])
                        eng = nc.vector if i in DVE_ROWS else nc.gpsimd
                        eng.tensor_mul(out=c_blk, in0=c_blk, in1=f_blk)

                    # sum the 25 product planes (q innermost, stride Wd)
                    red_in = bass.AP(ct.tensor, ct.offset,
                                     [ct.ap[0], (1, Wd), (Wd, K2)])
                    nc.vector.tensor_reduce(out=acc, in_=red_in,
                                            axis=mybir.AxisListType.X,
                                            op=mybir.AluOpType.add)

                    # out = W * pred
                    nc.vector.tensor_mul(out=acc, in0=acc, in1=wt)
                    nc.sync.dma_start(out=out_d[s, y0:y0 + P, :], in_=acc)

    nc.finalize()
    return nc


def _make_runner():
    """Build the per-core Bass program and a persistent jitted 8-core SPMD
    executor (mirrors bass_utils.run_bass_kernel_spmd's axon path, but keeps
    the jitted function alive so repeat calls don't recompile)."""
    import jax
    from jax.sharding import Mesh, PartitionSpec, NamedSharding
    from jax.experimental.shard_map import shard_map
    from concourse import bass2jax

    bass2jax.install_neuronx_cc_hook()
    nc = _build_program()

    partition_name = (nc.partition_id_tensor.name
                      if nc.partition_id_tensor is not None else None)
    in_names, out_names, out_avals = [], [], []
    for alloc in nc.m.functions[0].allocations:
        if not isinstance(alloc, mybir.MemoryLocationSet):
            continue
        name = alloc.memorylocations[0].name
        if alloc.kind == "ExternalInput":
            if name != partition_name:
                in_names.append(name)
        elif alloc.kind == "ExternalOutput":
            out_names.append(name)
            out_avals.append(jax.core.ShapedArray(tuple(alloc.tensor_shape),
                                                  mybir.dt.np(alloc.dtype)))
    n_params = len(in_names)
    all_in_names = in_names + out_names
    if partition_name is not None:
        all_in_names = all_in_names + [partition_name]

    def _body(*args):
        operands = list(args)
        if partition_name is not None:
            operands.append(bass2jax.partition_id_tensor())
        outs = bass2jax._bass_exec_p.bind(
            *operands,
            out_avals=tuple(out_avals),
            in_names=tuple(all_in_names),
            out_names=tuple(out_names),
            lowering_input_output_aliases=(),
            sim_require_finite=True,
            sim_require_nnan=True,
            nc=nc,
        )
        return tuple(outs)

    devices = jax.devices()[:N_CORES]
    mesh = Mesh(np.asarray(devices), ("core",))
    spec = PartitionSpec("core")
    n_outs = len(out_names)
    fn = jax.jit(
        shard_map(_body, mesh=mesh, in_specs=(spec,) * (n_params + n_outs),
                  out_specs=(spec,) * n_outs, check_rep=False),
        keep_unused=True,
    )
    sharding = NamedSharding(mesh, spec)
    return fn, in_names, out_names, out_avals, sharding


def _get_runner():
    global _RUNNER
    if _RUNNER is None:
        _RUNNER = _make_runner()
    return _RUNNER


def _pack_inputs(frames, core, w):
    """Full inputs -> concatenated per-core global arrays keyed by DRAM name."""
    frames = np.ascontiguousarray(np.asarray(frames, dtype=np.float32))
    core = np.ascontiguousarray(np.asarray(core, dtype=np.float32))
    w = np.ascontiguousarray(np.asarray(w, dtype=np.float32))

    frames_f = frames.reshape(B * N, H, Wd)
    fpad = np.zeros((B * N, HP, WP), dtype=np.float32)
    fpad[:, 2:2 + H, 2:2 + Wd] = frames_f
    core_f = core.reshape(B * N, K2, H, Wd)
    w_f = w.reshape(B * N, H, Wd)

    # shard m = 2c + s; global concat along axis 0 gives each device its pair
    return {
        "framesp": fpad,
        "corex": core_f,
        "w": w_f,
    }


def kernel(**inputs):
    import jax

    fn, in_names, out_names, out_avals, sharding = _get_runner()
    packed = _pack_inputs(inputs["frames"], inputs["core"], inputs["W"])
    args = [jax.device_put(packed[name], sharding) for name in in_names]
    zeros = [jax.device_put(
        np.zeros((N_CORES * a.shape[0],) + tuple(a.shape[1:]), a.dtype), sharding)
        for a in out_avals]
    outs = fn(*args, *zeros)
    out = np.asarray(outs[out_names.index("out")])
    return out.reshape(B, N, H, Wd)


def benchmark(inputs, iters=10):
    """Return best wall-clock ns per 8-core kernel execution (inputs resident
    on device, compile excluded)."""
    import jax, time

    fn, in_names, out_names, out_avals, sharding = _get_runner()
    packed = _pack_inputs(inputs["frames"], inputs["core"], inputs["W"])
    args = [jax.device_put(packed[name], sharding) for name in in_names]
    zeros = [jax.device_put(
        np.zeros((N_CORES * a.shape[0],) + tuple(a.shape[1:]), a.dtype), sharding)
        for a in out_avals]
    jax.block_until_ready(args)
    jax.block_until_ready(zeros)
    jax.block_until_ready(fn(*args, *zeros))  # warm
    best = float("inf")
    for _ in range(iters):
        t0 = time.perf_counter()
        jax.block_until_ready(fn(*args, *zeros))
        best = min(best, time.perf_counter() - t0)
    return int(best * 1e9)

